# revision 33
# baseline (speedup 1.0000x reference)
"""Trainium2 Bass kernel for a pre-norm transformer encoder block.

B=8 batches sharded 1 per NeuronCore (data parallel, no collectives).
Per-core math (S=1024, D=1024, H=16, DK=64, DFF=4096), all fp32 I/O:
    x = x + MHA(LN1(x));  out = x + FFN(LN2(x))

v7 strategy (HW-trace driven, from the 585us v3 baseline; loop-1 trace
now ~513us, chained steady state ~484us/iter at 84-86% PE occupancy):
  - V-projection bias folded into bo on the host (softmax rows sum to 1,
    so A(V+1bv^T)Wo^T == AVWo^T + 1(Wo@bv)^T): the v stage becomes a
    pure Act-engine descale, off the DVE which paces the LN1+v phase.
  - Softmax denominator: a second ones-lhsT fp8-DR GEMM writes the
    denominator onto 64 base-0 partitions of its own psum bank, so one
    reciprocal_approx_fast (5x faster than DVE reciprocal, ~18 bits)
    covers every partition the avT multiply needs -- no partition
    broadcast, no DRAM bounce, and the PE never waits on the DVE.
    Replaces v3's 3.3us full-precision reciprocal + double DRAM bounce
    (86 PE gaps, 139us idle, p-state drops to 1.2GHz).
  - All activation funcs (Exp/Ln/Relu/Copy) pinned to the one act table
    that holds them all (natural_log_exp_and_others): v3 flip-flopped
    tables around every layernorm, 31 x 1283ns ACT_TABLE_LOADs.
  - qT in fp16, kT e4m3 (mixed-dtype scores matmul costs the same as
    fp8 at K=64) and exp emitted as e4m3 with a 1/32 scale folded into
    the exp bias (e5m2's 2-bit mantissa was the largest attention error
    term).  FFN stays fp16: measured on HW, a DoubleRow fp8 matmul
    takes the same time as a half-K fp16 matmul (2x contraction per
    instruction, NOT the cost model's further 2x), so fp8 FFN tricks
    are cost-neutral while real-slab fp8 busts the 2e-2 gate.
  - Weight-stack DMAs issued from the idle GpSimd queue (v3 used the
    Scalar queue, serializing 1.4us dispatches against the exp chain).
  - q-half-2 scores/exp pairs ride a 2-deep queue threaded through the
    half-1 wo/LN2 phase, FFN1 and FFN2, so the Act-engine exp and the
    LN2 DVE chains hide under PE GEMMs end to end.
"""

import sys

import numpy as np

try:
    import concourse.bass as bass  # noqa: F401
except ImportError:
    sys.path.insert(0, "/opt/trn_rl_repo")

import ml_dtypes

import concourse.bass as bass
import concourse.bacc as bacc_mod
import concourse.tile as tile
from concourse import bacc, mybir
from concourse.bass_utils import run_bass_kernel_spmd
from concourse.hw_specs import get_activation_tables as _orig_act_tables
from concourse.masks import make_identity

P = 128
S = 1024
D = 1024
H = 16
DK = 64
DFF = 4096
EPS = 1e-5
ST = S // P    # 8 s-tiles
DT = D // P    # 8 d-tiles
FT = DFF // P  # 32 dff-tiles
VAR_CORR = D / (D - 1)  # torch.var ddof=1 correction on bn population var
WS = 32.0      # host-side e4m3 weight scale for wq/wk/wv/wo/w1
ESC = 1.0 / 32.0  # exp output scale (folded into exp bias); e4m3-safe
AVS = 4.0      # attn-out scale carried into the wo GEMM

F32 = mybir.dt.float32
F16 = mybir.dt.float16
BF16 = mybir.dt.bfloat16
E4 = mybir.dt.float8e4
AF = mybir.ActivationFunctionType
ALU = mybir.AluOpType
DR = mybir.MatmulPerfMode.DoubleRow

PIN_TABLE = "natural_log_exp_and_others"


def _pinned_act_tables(arch):
    """All funcs present in the pinned table resolve only to it, so the
    table-load fixpoint pass emits a single load for the whole kernel."""
    tabs = _orig_act_tables(arch)
    pin_funcs = tabs[PIN_TABLE]
    return {
        name: (funcs if name == PIN_TABLE else funcs - pin_funcs)
        for name, funcs in tabs.items()
    }


def build_nc(loop=1):
    nc = bacc.Bacc("TRN2", target_bir_lowering=False, debug=True)

    x_d = nc.dram_tensor("x", [S, D], F32, kind="ExternalInput")
    wqt_d = nc.dram_tensor("wqt", [D, D], E4, kind="ExternalInput")
    wkt_d = nc.dram_tensor("wkt", [D, D], E4, kind="ExternalInput")
    wvt_d = nc.dram_tensor("wvt", [D, D], E4, kind="ExternalInput")
    wot_d = nc.dram_tensor("wot", [D, D], E4, kind="ExternalInput")
    w1t_d = nc.dram_tensor("w1t", [D, DFF], F16, kind="ExternalInput")
    w2t_d = nc.dram_tensor("w2t", [DFF, D], F16, kind="ExternalInput")
    bq_d = nc.dram_tensor("bq", [D], F32, kind="ExternalInput")
    bk_d = nc.dram_tensor("bk", [D], F32, kind="ExternalInput")
    bo_d = nc.dram_tensor("bo", [D], BF16, kind="ExternalInput")
    b1_d = nc.dram_tensor("b1", [DFF], F32, kind="ExternalInput")
    b2_d = nc.dram_tensor("b2", [D], BF16, kind="ExternalInput")
    # [g1, be1, g2, be2]
    lnp_d = nc.dram_tensor("lnp", [4], F32, kind="ExternalInput")
    out_d = nc.dram_tensor("out", [S, D], F32, kind="ExternalOutput")
    x2_d = nc.dram_tensor("x2buf", [S, D], F32)  # post-attn residual scratch
    # chain buffers split by q-half: iteration i+1's phase A on half-1 rows
    # can start as soon as iteration i's half-1 FFN2 lands
    chain = [[nc.dram_tensor(f"chain{i}h{h}", [S // 2, D], F32)
              for h in range(2)] for i in range(2)] if loop > 1 else []

    def bcast_dram(ap1d, n):
        # 1D DRAM vector broadcast to all 128 partitions
        return bass.AP(tensor=ap1d.tensor, offset=ap1d.offset, ap=[[0, P], [1, n]])

    from contextlib import ExitStack

    with tile.TileContext(nc) as tc:
        with ExitStack() as ctx:
            pool = lambda *a, **k: ctx.enter_context(tc.tile_pool(*a, **k))
            singles = pool(name="singles", bufs=1)
            p_small = pool(name="small", bufs=4)
            p_x2t = pool(name="x2t", bufs=2)
            ps_mm = pool(name="psmm", bufs=4, space="PSUM")
            ps_sc = pool(name="pssc", bufs=2, space="PSUM")
            # ---- constants ----
            ident = singles.tile([P, P], F16)
            make_identity(nc, ident)
            # lhsT for the K=1 softmax-denominator broadcast matmul
            ones_bf = singles.tile([1, DK], BF16)
            nc.vector.memset(ones_bf, 1.0)
            lnp = singles.tile([P, 4], F32)  # g1, be1, g2, be2 bcast to all parts
            nc.scalar.dma_start(out=lnp, in_=bcast_dram(lnp_d[:], 4))
            bqc = singles.tile([P, DT], F32)  # per-partition bias cols per d-tile
            nc.scalar.dma_start(out=bqc, in_=bq_d[:].rearrange("(t p) -> p t", p=P))
            bkc = singles.tile([P, DT], F32)
            nc.scalar.dma_start(out=bkc, in_=bk_d[:].rearrange("(t p) -> p t", p=P))
            b1c = singles.tile([P, FT], F32)
            nc.scalar.dma_start(out=b1c, in_=b1_d[:].rearrange("(t p) -> p t", p=P))
            bo_bc = singles.tile([P, D], BF16)
            nc.scalar.dma_start(out=bo_bc, in_=bcast_dram(bo_d[:], D))
            epsc = singles.tile([P, 1], F32)
            nc.vector.memset(epsc, float(EPS))
            lnesc = singles.tile([P, 1], F32)  # exp bias: ln(ESC)
            nc.vector.memset(lnesc, float(np.log(ESC)))
            zeroc = singles.tile([P, 1], F32)
            nc.vector.memset(zeroc, 0.0)

            def layernorm_tile(xt, g_col, be_col, dst_pool, out_dt,
                               affine_on_act=False):
                """LN over free dim D for one natural s-tile; returns tile."""
                st = p_small.tile([P, 2, 6], F32, name="bnst")
                nc.vector.bn_stats(out=st[:, 0, :], in_=xt[:, 0:512])
                nc.vector.bn_stats(out=st[:, 1, :], in_=xt[:, 512:1024])
                mv = p_small.tile([P, 2], F32, name="bnmv")
                nc.vector.bn_aggr(out=mv, in_=st)
                # rstd = exp(-0.5*ln(vc*var+eps)); Ln+Exp both live in the
                # pinned act table, so no table reloads
                lnv = p_small.tile([P, 1], F32, name="lnv")
                nc.scalar.activation(
                    out=lnv, in_=mv[:, 1:2], func=AF.Ln,
                    bias=epsc, scale=float(VAR_CORR),
                )
                rstd = p_small.tile([P, 1], F32, name="rstd")
                nc.scalar.activation(
                    out=rstd, in_=lnv, func=AF.Exp,
                    bias=zeroc, scale=-0.5,
                )
                gmul = p_small.tile([P, 1], F32, name="gmul")
                nc.vector.tensor_mul(gmul, rstd, g_col)
                mg = p_small.tile([P, 1], F32, name="mg")
                nc.vector.tensor_mul(mg, mv[:, 0:1], gmul)
                bias2 = p_small.tile([P, 1], F32, name="bias2")
                nc.vector.tensor_sub(bias2, be_col, mg)
                ht = dst_pool.tile([P, D], out_dt, name="hnat")
                if affine_on_act:
                    # phase A is DVE-paced; Act has slack there
                    nc.scalar.activation(
                        out=ht, in_=xt, func=AF.Identity,
                        bias=bias2, scale=gmul,
                    )
                else:
                    nc.vector.tensor_scalar(
                        out=ht, in0=xt, scalar1=gmul, scalar2=bias2,
                        op0=ALU.mult, op1=ALU.add,
                    )
                return ht

            for _it in range(loop):
                if _it == 0:
                    x_src_ap = lambda r0, r1: x_d[r0:r1, :]
                else:
                    cin = chain[_it % 2]
                    x_src_ap = (lambda cin: lambda r0, r1:
                                cin[r0 // 512][r0 % 512:r0 % 512 + (r1 - r0), :])(cin)
                if _it == loop - 1:
                    out_dst_ap = lambda r0, r1, c0, c1: out_d[r0:r1, c0:c1]
                else:
                    cout = chain[(_it + 1) % 2]
                    out_dst_ap = (lambda cout: lambda r0, r1, c0, c1:
                                  cout[r0 // 512][r0 % 512:r0 % 512 + (r1 - r0), c0:c1])(cout)
                with ExitStack() as attn_ctx:
                    apool = lambda *a, **k: attn_ctx.enter_context(tc.tile_pool(*a, **k))
                    p_qT = apool(name="qT", bufs=1)
                    p_kT = apool(name="kT", bufs=1)
                    p_vaug = apool(name="vaug", bufs=1)
                    p_avT = apool(name="avT", bufs=1)
                    qT = p_qT.tile([P, DT, S], F16, name="qT")
                    kT = p_kT.tile([P, DT, S], E4, name="kT")
                    vaug = p_vaug.tile([P, ST, H, DK + 1], E4, name="vaug")

                    p_e = apool(name="e", bufs=2)
                    p_rb = apool(name="rb", bufs=2)
                    p_denb = apool(name="denb", bufs=2)

                    def emit_scores_exp(pr, half):
                        sl = slice(half * 512, (half + 1) * 512)
                        e8 = p_e.tile([P, ST, 2, 512], E4, name="e")
                        for t in range(ST):
                            ps01 = ps_sc.tile([P, 2, 512], F32, name="sc")
                            # heads 2pr (parts 0:64) / 2pr+1 (64:128):
                            # disjoint PE row groups, adjacent emit
                            nc.tensor.matmul(
                                ps01[:, 0, :],
                                kT[0:DK, pr, t * P:(t + 1) * P],
                                qT[0:DK, pr, sl],
                                start=True, stop=True,
                            )
                            nc.tensor.matmul(
                                ps01[:, 1, :],
                                kT[DK:P, pr, t * P:(t + 1) * P],
                                qT[DK:P, pr, sl],
                                start=True, stop=True,
                            )
                            # e = ESC * exp(s / sqrt(DK)), both heads at once
                            nc.scalar.activation(
                                out=e8[:, t, :, :],
                                in_=ps01,
                                func=AF.Exp,
                                bias=lnesc,
                                scale=float(1.0 / np.sqrt(DK)),
                            )
                        return e8

                    def emit_av(pr, half, e8, avT):
                        """AV GEMMs for both heads of pair pr.  The softmax
                        denominator rides along as the augmented 65th V row
                        (cost-free); it is copied to bf16 SBUF and partition-
                        broadcast by a K=1 ones-matmul into a base-0 psum
                        bank, where the wide approx-reciprocal is valid."""
                        psas, denbs = [], []
                        for h2i in range(2):
                            head = 2 * pr + h2i
                            psa = ps_mm.tile([P, 512], F32, name="mm")
                            for t in range(ST // 2):
                                nc.tensor.matmul(
                                    psa[0:DK + 1],
                                    vaug[:, 2 * t:2 * t + 2, head, :],
                                    e8[:, 2 * t:2 * t + 2, h2i, :],
                                    start=(t == 0), stop=(t == ST // 2 - 1),
                                    perf_mode=DR,
                                )
                            denb = p_denb.tile([1, 512], BF16, name="denb")
                            nc.vector.tensor_copy(denb, psa[DK:DK + 1, :])
                            psas.append(psa)
                            denbs.append(denb)
                        denps = []
                        for h2i in range(2):
                            dps = ps_mm.tile([P, 512], F32, name="mm")
                            nc.tensor.matmul(
                                dps[0:DK, :], ones_bf, denbs[h2i],
                                start=True, stop=True,
                            )
                            denps.append(dps)
                        for h2i in range(2):
                            rb = p_rb.tile([DK, 512], F32, name="rb")
                            nc.vector.reciprocal_approx_fast(
                                out=rb, in_=denps[h2i][0:DK, :])
                            nc.vector.tensor_mul(
                                avT[h2i * DK:(h2i + 1) * DK, pr, :],
                                psas[h2i][0:DK, :],
                                rb,
                            )

                    avT1 = p_avT.tile([P, DT, 512], E4, name="avT")

                    # ---- LN1 + transpose + qkv projections ----
                    with ExitStack() as c12:
                        bpool = lambda *a, **k: c12.enter_context(tc.tile_pool(*a, **k))
                        p_xs = bpool(name="xs", bufs=2)
                        p_h1 = bpool(name="hnat", bufs=2)
                        p_hT = bpool(name="hTa", bufs=1)
                        p_wlhs = bpool(name="wlhs", bufs=2)
                        p_wrhs = bpool(name="wrhs", bufs=2)
                        h1T = p_hT.tile([P, DT, S], E4, name="hT")
                        # denominator column: 1/(WS*AVS) is an exact e4m3
                        # subnormal, so avT lands at x128 (e4 sweet spot)
                        nc.vector.memset(vaug[:, :, :, DK:DK + 1], 1.0 / (WS * AVS))
                        wv_stacks = []
                        for c in range(2):
                            slv = slice(c * 512, (c + 1) * 512)
                            wv_s = p_wrhs.tile([P, DT, 512], E4, name="wrhs")
                            nc.gpsimd.dma_start(
                                out=wv_s,
                                in_=wvt_d[:, slv].rearrange(
                                    "(kt p) o -> p kt o", p=P),
                            )
                            wv_stacks.append(wv_s)
                        # prefetch the q/k weight stacks now: their 1MB
                        # transfers hide under the whole LN1+v phase instead
                        # of stalling the first qk GEMMs (~8us on the trace)
                        wq_s = p_wlhs.tile([P, DT, D], E4, name="wlhs")
                        nc.gpsimd.dma_start(
                            out=wq_s,
                            in_=wqt_d[:].rearrange("(kt p) o -> p kt o", p=P),
                        )
                        wk_s = p_wlhs.tile([P, DT, D], E4, name="wlhs")
                        nc.gpsimd.dma_start(
                            out=wk_s,
                            in_=wkt_d[:].rearrange("(kt p) o -> p kt o", p=P),
                        )
                        for i in range(ST):
                            xt = p_xs.tile([P, D], F32, name="xs")
                            nc.sync.dma_start(out=xt, in_=x_src_ap(i * P, (i + 1) * P))
                            h1 = layernorm_tile(xt, lnp[:, 0:1], lnp[:, 1:2], p_h1, F16)
                            pst = ps_sc.tile([P, DT, P], F16, name="sc")
                            for j in range(DT):
                                nc.tensor.transpose(
                                    pst[:, j, :], h1[:, j * P:(j + 1) * P], ident)
                            nc.scalar.copy(
                                h1T[:, :, i * P:(i + 1) * P], pst)
                            # v projection for this s-tile right away: keeps
                            # the PE fed while the next LN1 runs on DVE
                            for c in range(2):
                                slv = slice(c * 512, (c + 1) * 512)
                                psv = ps_mm.tile([P, 512], F32, name="mm")
                                for k in range(DT // 2):
                                    nc.tensor.matmul(
                                        psv,
                                        h1T[:, 2 * k:2 * k + 2, i * P:(i + 1) * P],
                                        wv_stacks[c][:, 2 * k:2 * k + 2, :],
                                        start=(k == 0), stop=(k == DT // 2 - 1),
                                        perf_mode=DR,
                                    )
                                nc.scalar.activation(
                                    out=vaug[:, i, c * 8:(c + 1) * 8, 0:DK],
                                    in_=psv.rearrange("p (h d) -> p h d", d=DK),
                                    func=AF.Copy, bias=0.0, scale=1.0 / WS,
                                )

                        def emit_qk(j):  # noqa: E306
                            for (w_s, dstT, bc) in ((wq_s, qT, bqc), (wk_s, kT, bkc)):
                                psq = [ps_mm.tile([P, 512], F32, name="mm")
                                       for _ in range(2)]
                                for k in range(DT // 2):
                                    kk = slice(2 * k, 2 * k + 2)
                                    for c in range(2):
                                        sl = slice(c * 512, (c + 1) * 512)
                                        nc.tensor.matmul(
                                            psq[c],
                                            w_s[:, kk, j * P:(j + 1) * P],
                                            h1T[:, kk, sl],
                                            start=(k == 0), stop=(k == DT // 2 - 1),
                                            perf_mode=DR,
                                        )
                                for c in range(2):
                                    sl = slice(c * 512, (c + 1) * 512)
                                    nc.vector.tensor_scalar(
                                        out=dstT[:, j, sl], in0=psq[c],
                                        scalar1=1.0 / WS, scalar2=bc[:, j:j + 1],
                                        op0=ALU.mult, op1=ALU.add,
                                    )

                        # interleave q-half-1 head pairs behind each q/k tile:
                        # exp(pair j) overlaps q/k proj of tile j+1
                        pend0 = None
                        for j in range(DT):
                            emit_qk(j)
                            if pend0 is not None:
                                emit_av(pend0[0], 0, pend0[1], avT1)
                            pend0 = (j, emit_scores_exp(j, 0))

                    # preload w2 during the head loop (no deps)
                    p_wo = apool(name="wo", bufs=2)
                    p_b2b = apool(name="b2b", bufs=1)
                    b2_bc2 = p_b2b.tile([P, D], BF16, name="b2b")
                    nc.scalar.dma_start(out=b2_bc2, in_=bcast_dram(b2_d[:], D))
                    p_w2s = apool(name="w2s", bufs=2)
                    w2_stacks = []
                    for c in range(2):
                        sl = slice(c * 512, (c + 1) * 512)
                        w2_s = p_w2s.tile([P, FT, 512], F16, name="w2s")
                        nc.gpsimd.dma_start(
                            out=w2_s[:, 0:FT // 2, :],
                            in_=w2t_d[0:DFF // 2, sl].rearrange(
                                "(ft p) o -> p ft o", p=P),
                        )
                        nc.sync.dma_start(
                            out=w2_s[:, FT // 2:FT, :],
                            in_=w2t_d[DFF // 2:DFF, sl].rearrange(
                                "(ft p) o -> p ft o", p=P),
                        )
                        w2_stacks.append(w2_s)

                    # ---- per q-half: heads -> wo+residual -> LN2 -> FFN ----
                    # q-half-2's scores/exp interleave into q-half-1's FFN1
                    # and FFN2 loops so the Act-engine exp hides under PE GEMMs
                    p_xr = apool(name="xr", bufs=2)  # shared x/x2 reload pool
                    p_h2 = apool(name="hnat2", bufs=1)
                    p_h2d = apool(name="h2d", bufs=1)
                    p_wlhs2 = apool(name="wlhs2", bufs=2)
                    p_ff1 = apool(name="ff1", bufs=1)
                    p_stage = apool(name="stage2", bufs=1)

                    def emit_wo_ln2(half, avT, hook=None):
                        """wo GEMM + residual + LN2 + fp16 transpose of h2."""
                        qtiles = range(half * (ST // 2), (half + 1) * (ST // 2))
                        h2T = p_h2d.tile([P, DT, 512], F16, name="h2T")
                        wo_s = [None, None]
                        for c in range(2):
                            slc = slice(c * 512, (c + 1) * 512)
                            wo_s[c] = p_wo.tile([P, DT, 512], E4, name="wo")
                            nc.gpsimd.dma_start(
                                out=wo_s[c],
                                in_=wot_d[:, slc].rearrange(
                                    "(kt p) o -> p kt o", p=P),
                            )
                        for i in qtiles:
                            ii = i - half * (ST // 2)
                            xrs = []
                            for c in range(2):
                                xr = p_xr.tile([P, 512], F32, name="xr")
                                src = x_src_ap(i * P, (i + 1) * P)
                                nc.sync.dma_start(
                                    out=xr, in_=src[:, c * 512:(c + 1) * 512])
                                xrs.append(xr)
                            x2t = p_x2t.tile([P, D], F32, name="x2t")
                            pso = [ps_mm.tile([P, 512], F32, name="mm")
                                   for _ in range(2)]
                            for jj in range(DT // 2):
                                kk = slice(2 * jj, 2 * jj + 2)
                                for c in range(2):
                                    slc = slice(c * 512, (c + 1) * 512)
                                    nc.tensor.matmul(
                                        pso[c],
                                        avT[:, kk, ii * P:(ii + 1) * P],
                                        wo_s[c][:, kk, :],
                                        start=(jj == 0),
                                        stop=(jj == DT // 2 - 1),
                                        perf_mode=DR,
                                    )
                            for c in range(2):
                                slc = slice(c * 512, (c + 1) * 512)
                                # x2 = pso/(WS*AVS) + bo + xr
                                nc.vector.scalar_tensor_tensor(
                                    out=x2t[:, slc], in0=pso[c],
                                    scalar=1.0 / (WS * WS * AVS),  # avT = (WS*AVS)*av
                                    in1=bo_bc[:, slc],
                                    op0=ALU.mult, op1=ALU.add,
                                )
                                nc.vector.tensor_add(
                                    x2t[:, slc], x2t[:, slc], xrs[c]
                                )
                            nc.sync.dma_start(
                                out=x2_d[i * P:(i + 1) * P, :], in_=x2t
                            )
                            # LN2 + transpose + hi/lo split for this s-tile
                            h2t = layernorm_tile(
                                x2t, lnp[:, 2:3], lnp[:, 3:4], p_h2, F16)
                            pst = ps_sc.tile([P, DT, P], F16, name="sc")
                            for j in range(DT):
                                nc.tensor.transpose(
                                    pst[:, j, :], h2t[:, j * P:(j + 1) * P], ident)
                            nc.vector.tensor_copy(
                                h2T[:, :, ii * P:(ii + 1) * P], pst)
                            if hook is not None:
                                hook(ii)
                        return h2T

                    def emit_w1_group(g):
                        # 4 f-tiles of w1 in one DMA
                        w1_s = p_wlhs2.tile([P, DT, 512], F16, name="wlhs2")
                        nc.gpsimd.dma_start(
                            out=w1_s,
                            in_=w1t_d[:, g * 512:(g + 1) * 512].rearrange(
                                "(kt p) o -> p kt o", p=P),
                        )
                        return w1_s

                    def emit_ffn1_tile(f, h2T, ff1, w1_s):
                        # relu+bias split DVE/Act: Act stays mostly free for exp
                        fo = (f % 4) * P
                        ps1 = ps_mm.tile([P, 512], F32, name="mm")
                        for k in range(DT):
                            nc.tensor.matmul(
                                ps1, w1_s[:, k, fo:fo + P], h2T[:, k, :],
                                start=(k == 0), stop=(k == DT - 1),
                            )
                        if f % 2 == 0:
                            nc.vector.tensor_scalar(
                                out=ff1[:, f, :], in0=ps1,
                                scalar1=b1c[:, f:f + 1], scalar2=0.0,
                                op0=ALU.add, op1=ALU.max,
                            )
                        else:
                            nc.scalar.activation(
                                out=ff1[:, f, :], in_=ps1, func=AF.Relu,
                                bias=b1c[:, f:f + 1],
                            )

                    def emit_ffn2(half, ff1, hook=None, subset=None):
                        qtiles = [half * (ST // 2) + k for k in
                                  (subset if subset is not None
                                   else range(ST // 2))]
                        for ii, i in zip((subset if subset is not None
                                          else range(ST // 2)), qtiles):
                            ps2 = [ps_mm.tile([P, 512], F32, name="mm")
                                   for _ in range(2)]
                            for f in range(FT):
                                for c in range(2):
                                    nc.tensor.matmul(
                                        ps2[c], ff1[:, f, ii * P:(ii + 1) * P],
                                        w2_stacks[c][:, f, :],
                                        start=(f == 0), stop=(f == FT - 1),
                                    )
                            for c in range(2):
                                slc = slice(c * 512, (c + 1) * 512)
                                x2r = p_xr.tile([P, 512], F32, name="xr")
                                nc.sync.dma_start(
                                    out=x2r, in_=x2_d[i * P:(i + 1) * P, slc])
                                stg = p_stage.tile([P, 512], F32, name="stage")
                                nc.vector.tensor_add(stg, ps2[c], b2_bc2[:, slc])
                                nc.vector.tensor_add(stg, stg, x2r)
                                nc.sync.dma_start(
                                    out=out_dst_ap(i * P, (i + 1) * P,
                                                   slc.start, slc.stop),
                                    in_=stg
                                )
                            if hook is not None:
                                hook(ii)

                    # -- q-half 1 tail: last pending av pair --
                    emit_av(pend0[0], 0, pend0[1], avT1)
                    avT2 = p_avT.tile([P, DT, 512], E4, name="avT")
                    # q-half-2 pairs ride a 2-deep queue: scores/exp start in
                    # the wo/LN2 phase (keeping the PE fed while the LN chain
                    # runs on DVE/Act), but their AV GEMMs only flush once the
                    # B1 wo GEMMs -- the last avT1 readers -- are emitted,
                    # since avT2 reuses avT1's single pool buffer
                    pends = []

                    def push_pair(pr):
                        if len(pends) == 2:
                            pp = pends.pop(0)
                            emit_av(pp[0], 1, pp[1], avT2)
                        pends.append((pr, emit_scores_exp(pr, 1)))

                    def flush_pair():
                        if pends:
                            pp = pends.pop(0)
                            emit_av(pp[0], 1, pp[1], avT2)

                    h2T1 = emit_wo_ln2(0, avT1,
                                       hook=lambda ii: push_pair(ii // 2)
                                       if ii in (0, 2) else None)
                    # -- q-half 1 FFN1 hosts pairs 2-6 --
                    ff1a = p_ff1.tile([P, FT, 512], F16, name="ff1")
                    w1g = None
                    for f in range(FT):
                        if f % 8 == 0:
                            w1g = emit_w1_group(f // 8 * 2)
                        elif f % 8 == 4:
                            w1g = emit_w1_group(f // 8 * 2 + 1)
                        if f in (2, 7, 12, 17, 22):
                            push_pair(2 + (f - 2) // 5)
                        emit_ffn1_tile(f, h2T1, ff1a, w1g)

                    # -- q-half 1 FFN2 tiles 0-2 host pair 7 + last avs --
                    def ffn2_hook(ii):
                        if ii == 0:
                            push_pair(7)
                        else:
                            flush_pair()

                    emit_ffn2(0, ff1a, hook=ffn2_hook, subset=[0, 1, 2])
                    flush_pair()
                    # -- q-half 2 wo+LN2 before the last ffn2 tile of half 1,
                    # so its DVE/Act chain hides under that tile's GEMMs --
                    h2T2 = emit_wo_ln2(1, avT2)
                    emit_ffn2(0, ff1a, subset=[3])
                    ff1b = p_ff1.tile([P, FT, 512], F16, name="ff1")
                    w1g = None
                    for f in range(FT):
                        if f % 8 == 0:
                            w1g = emit_w1_group(f // 8 * 2)
                        elif f % 8 == 4:
                            w1g = emit_w1_group(f // 8 * 2 + 1)
                        emit_ffn1_tile(f, h2T2, ff1b, w1g)
                    emit_ffn2(1, ff1b)

    # pin every activation to the one table holding Exp/Ln/Relu/Copy so the
    # fixpoint pass emits a single ACT_TABLE_LOAD (scoped: restored after)
    bacc_mod.get_activation_tables = _pinned_act_tables
    try:
        nc.finalize()  # Bacc: run compile passes (register allocation etc.)
    finally:
        bacc_mod.get_activation_tables = _orig_act_tables
    return nc


_NC_CACHE = None


def get_nc():
    global _NC_CACHE
    if _NC_CACHE is None:
        _NC_CACHE = build_nc()
    return _NC_CACHE


def make_in_maps(x, wq, bq, wk, bk, wv, bv, wo, bo, w1, b1, w2, b2,
                 g1, be1, g2, be2):
    """x: [8, 1024, 1024]; returns per-core input maps."""
    f32c = lambda a: np.ascontiguousarray(np.asarray(a), dtype=np.float32)
    e4c = lambda a: np.ascontiguousarray(
        (np.asarray(a, dtype=np.float32).T * WS).astype(ml_dtypes.float8_e4m3))
    f16c = lambda a: np.ascontiguousarray(
        np.asarray(a, dtype=np.float32).T.astype(np.float16))
    shared = {
        "wqt": e4c(wq),
        "wkt": e4c(wk),
        "wvt": e4c(wv),
        "wot": e4c(wo),
        "w1t": f16c(w1),
        "w2t": f16c(w2),
        "bq": f32c(bq), "bk": f32c(bk),
        # softmax rows sum to 1, so the V bias passes through attention as
        # the constant wo @ bv -- folded into bo here instead of on-device
        "bo": np.ascontiguousarray(
            (np.asarray(bo, np.float32)
             + np.asarray(wo, np.float32) @ np.asarray(bv, np.float32)
             ).astype(ml_dtypes.bfloat16)),
        "b1": f32c(b1), "b2": np.ascontiguousarray(np.asarray(b2, np.float32).astype(ml_dtypes.bfloat16)),
        "lnp": np.array(
            [np.float32(np.asarray(g1).reshape(-1)[0]),
             np.float32(np.asarray(be1).reshape(-1)[0]),
             np.float32(np.asarray(g2).reshape(-1)[0]),
             np.float32(np.asarray(be2).reshape(-1)[0])],
            dtype=np.float32,
        ),
    }
    x = np.asarray(x, dtype=np.float32)
    return [dict(shared, x=np.ascontiguousarray(x[i])) for i in range(8)]


def kernel(x, src_mask, wq, bq, wk, bk, wv, bv, wo, bo,
           w1, b1, w2, b2, g1, be1, g2, be2):
    # src_mask is all-ones and has no effect in the reference math.
    nc = get_nc()
    in_maps = make_in_maps(x, wq, bq, wk, bk, wv, bv, wo, bo,
                           w1, b1, w2, b2, g1, be1, g2, be2)
    res = run_bass_kernel_spmd(nc, in_maps, list(range(8))).results
    return np.stack([res[i]["out"] for i in range(8)], axis=0)
